# revision 10
# baseline (speedup 1.0000x reference)
"""Root-to-leaves TreeLSTM over a complete binary tree (depth 17, 131071 nodes,
feat=h=512), distributed over 8 TRN2 NeuronCores with zero inter-core
communication — fp8 DoubleRow edition.

Sharding identical to the bf16 baseline: each core owns one of the 8 subtrees
rooted at level 3; levels 0-3 replicated with XOR relabeling so one SPMD NEFF
serves all cores. Within a level the columns are [left-children |
right-children] so parent h/c state reads are contiguous slices.

Precision (CPU-sim validated, rel_err ~1.3e-2 vs 2e-2 gate):
- gate x-GEMM (20 M-tiles) and parent-h GEMM (20 M-tiles): fp8e4 (TRN E4M3)
  with perf_mode=DoubleRow (k-tile pairs [128,2,N]); weights pre-scaled by
  S=32 host-side, descaled via the activation's `scale`.
- px GEMM: bf16 (fp8 here alone costs 3.1e-2 — px feeds the output linearly,
  unsquashed by any sigmoid). Features are shipped twice: fp8 + bf16.
- h state fp8, c state fp32, elementwise fp32.
DoubleRow only for streams >= 128 cols (below that it disables FWL and loses).
"""

import os
import sys

sys.path.insert(0, "/opt/trn_rl_repo")

import numpy as np
import ml_dtypes
from contextlib import ExitStack

import concourse.bass as bass
import concourse.mybir as mybir
import concourse.tile as tile
from concourse import bacc

P = 128
KT = 4              # 512 / 128 contraction tiles
NKP = 2             # k-tile pairs for DoubleRow
H = 512
F = 512
DEPTH = 17
NCORES = 8
CHUNK = 512
M_IOFU = 20         # iofu M-tiles (2560/128)
M_ALL = 24          # + px M-tiles (512/128)
M_PX = 4
SPLIT_THRESH = 2048  # split last-2 levels when parent level exceeds this
S_W = 32.0          # host-side weight pre-scale before fp8 quantization
DR_MIN = 128        # min streamed cols for DoubleRow to win
BF16 = mybir.dt.bfloat16
F32 = mybir.dt.float32
FP8 = mybir.dt.float8e4
DR = mybir.MatmulPerfMode.DoubleRow
AF = mybir.ActivationFunctionType
ALU = mybir.AluOpType
np_bf16 = ml_dtypes.bfloat16
np_fp8 = ml_dtypes.float8_e4m3   # == mybir.dt.np(float8e4); TRN e4m3 max ±240


def _level_sizes(depth):
    # per-core column count per level: levels 0..3 replicated, >=4 core-private
    return [1 << d if d <= 3 else 1 << (d - 3) for d in range(depth)]


def _plan(depth):
    """Segment schedule. Each seg = (level, seg_start, seg_len) in within-level
    logical coords. The last level's parent level is split in halves when it
    would otherwise need >2048 state columns, interleaving the two subtrees to
    halve peak state SBUF."""
    Ns = _level_sizes(depth)
    off = [0]
    for n in Ns:
        off.append(off[-1] + n)
    segs = []
    split = depth >= 2 and Ns[-2] > SPLIT_THRESH
    if split:
        for d in range(depth - 2):
            segs.append((d, 0, Ns[d]))
        for h in range(2):
            segs.append((depth - 2, h * Ns[depth - 2] // 2, Ns[depth - 2] // 2))
            segs.append((depth - 1, h * Ns[depth - 1] // 2, Ns[depth - 1] // 2))
    else:
        segs = [(d, 0, Ns[d]) for d in range(depth)]
    stored = [s for s in segs if s[0] < depth - 1]
    store_cols = max(s[2] for s in stored) if stored else 1
    return Ns, off, segs, split, store_cols


def _host_levels(depth):
    return 4 if depth >= 6 else 0


def build_nc(depth=DEPTH):
    """Build the SPMD single-core Bass program (same NEFF for all 8 cores)."""
    Ns, off, segs, split, store_cols = _plan(depth)
    C = off[-1]
    HD = _host_levels(depth)

    nc = bacc.Bacc("TRN2", target_bir_lowering=False, debug=False)
    featsT = nc.declare_dram_parameter("featsT", [F, C], FP8, isOutput=False)
    featsB = nc.declare_dram_parameter("featsB", [F, C], BF16, isOutput=False)
    wxT = nc.declare_dram_parameter("wxT", [F, M_IOFU * P], FP8, isOutput=False)
    whT = nc.declare_dram_parameter("whT", [H, M_IOFU * P], FP8, isOutput=False)
    pxwT = nc.declare_dram_parameter("pxwT", [F, M_PX * P], BF16, isOutput=False)
    biasm = nc.declare_dram_parameter("biasm", [P, M_ALL], F32, isOutput=False)
    outT = nc.declare_dram_parameter("outT", [H, C], F32, isOutput=True)
    if HD:
        c_init = nc.declare_dram_parameter("c_init", [H, 1], F32,
                                           isOutput=False)
        h_init = nc.declare_dram_parameter("h_init", [H, 1], FP8,
                                           isOutput=False)
        c_init_r = c_init[:].rearrange("(a p) c -> p a c", p=P)
        h_init_r = h_init[:].rearrange("(a p) c -> p a c", p=P)

    featsT_r = featsT[:].rearrange("(a p) c -> p a c", p=P)
    featsB_r = featsB[:].rearrange("(a p) c -> p a c", p=P)
    wxT_r = wxT[:].rearrange("(a p) m -> p a m", p=P)
    whT_r = whT[:].rearrange("(a p) m -> p a m", p=P)
    pxwT_r = pxwT[:].rearrange("(a p) m -> p a m", p=P)
    outT_r = outT[:].rearrange("(a p) c -> p a c", p=P)
    inv_s = 1.0 / S_W

    with ExitStack() as ctx:
        tc = ctx.enter_context(tile.TileContext(nc))
        wpool = ctx.enter_context(tc.tile_pool(name="w", bufs=1))
        spool = ctx.enter_context(tc.tile_pool(name="state", bufs=1))
        fpool = ctx.enter_context(tc.tile_pool(name="feats", bufs=6))
        fbpool = ctx.enter_context(tc.tile_pool(name="featsb", bufs=6))
        pspool = ctx.enter_context(tc.tile_pool(name="ps", bufs=8, space="PSUM"))
        gpool = ctx.enter_context(tc.tile_pool(name="gates", bufs=14))
        xpool = ctx.enter_context(tc.tile_pool(name="px", bufs=6))
        tpool = ctx.enter_context(tc.tile_pool(name="tmp", bufs=9))
        opool = ctx.enter_context(tc.tile_pool(name="hf", bufs=4))

        wx_sb = wpool.tile([P, KT, M_IOFU * P], FP8, tag="wx")
        wh_sb = wpool.tile([P, KT, M_IOFU * P], FP8, tag="wh")
        pxw_sb = wpool.tile([P, KT, M_PX * P], BF16, tag="pxw")
        bias_sb = wpool.tile([P, M_ALL], F32, tag="bias")
        nc.sync.dma_start(wx_sb[:], wxT_r)
        nc.sync.dma_start(wh_sb[:], whT_r)
        nc.sync.dma_start(pxw_sb[:], pxwT_r)
        nc.sync.dma_start(bias_sb[:], biasm[:])

        # state double buffers: c fp32, hf fp8 (level d -> buffer d % 2)
        cst = [spool.tile([P, KT, store_cols], F32, tag=f"c{b}", name=f"c{b}")
               for b in (0, 1)]
        hst = [spool.tile([P, KT, store_cols], FP8, tag=f"h{b}", name=f"h{b}")
               for b in (0, 1)]
        if HD:
            ibuf = (HD - 1) % 2
            nc.sync.dma_start(cst[ibuf][:, :, 0:1], c_init_r)
            nc.sync.dma_start(hst[ibuf][:, :, 0:1], h_init_r)

        def v3(ap):
            # flat [P, w] chunk view -> [P, 2, w//2] b-major (left|right block)
            return ap.rearrange("p (b q) -> p b q", b=2)

        def mm8(ps_ap, w_sb, m, rhs, c0, w, first, last):
            """fp8 GEMM for M-tile m over rhs cols [c0, c0+w): DoubleRow pairs
            when the stream is wide enough, 4 plain k-tile matmuls if not."""
            if w >= DR_MIN:
                for kp in range(NKP):
                    nc.tensor.matmul(
                        ps_ap, w_sb[:, 2 * kp:2 * kp + 2, m * P:(m + 1) * P],
                        rhs[:, 2 * kp:2 * kp + 2, c0:c0 + w],
                        start=(first and kp == 0), stop=(last and kp == NKP - 1),
                        perf_mode=DR)
            else:
                for k in range(KT):
                    nc.tensor.matmul(
                        ps_ap, w_sb[:, k, m * P:(m + 1) * P],
                        rhs[:, k, c0:c0 + w],
                        start=(first and k == 0), stop=(last and k == KT - 1))

        def px_from(ps_px, t, ftb, w):
            """px GEMM (bf16) for t-tile via pxw: 4 k matmuls."""
            for k in range(KT):
                nc.tensor.matmul(
                    ps_px[:, :w], pxw_sb[:, k, t * P:(t + 1) * P],
                    ftb[:, k, :w], start=(k == 0), stop=(k == KT - 1))

        def elemwise(t, w, gates, ps_px, pc_ap, c_dst, h_dst, col0, bcast):
            """pc_ap/c_dst may be [P,2,w/2] broadcast/b-major views (bcast).
            The fp8 h-state write is a second e_+px add (on Pool, parallel to
            the fp32 hf add on DVE) so the next level's h-GEMM isn't serialized
            behind the output-path add."""
            gi_, go_, gf_, gu_, gr_ = gates
            small = w < 256  # latency-bound: keep chain on the faster DVE
            pool_e = nc.vector if small else nc.gpsimd
            px = xpool.tile([P, CHUNK], F32, tag="px")
            nc.vector.tensor_scalar_add(
                px[:, :w], ps_px[:, :w],
                bias_sb[:, M_IOFU + t:M_IOFU + t + 1])
            if pc_ap is None:
                nc.vector.tensor_mul(c_dst, gi_[:, :w], gu_[:, :w])
            else:
                t1 = tpool.tile([P, CHUNK], F32, tag="tmp")
                nc.vector.tensor_mul(t1[:, :w], gi_[:, :w], gu_[:, :w])
                t2 = tpool.tile([P, CHUNK], F32, tag="tmp")
                if bcast:
                    nc.vector.tensor_mul(v3(t2[:, :w]), v3(gf_[:, :w]), pc_ap)
                else:
                    nc.vector.tensor_mul(t2[:, :w], gf_[:, :w], pc_ap)
                nc.vector.tensor_add(c_dst,
                                     v3(t1[:, :w]) if bcast else t1[:, :w],
                                     v3(t2[:, :w]) if bcast else t2[:, :w])
            tc_ = tpool.tile([P, CHUNK], F32, tag="tmp")
            nc.scalar.activation(v3(tc_[:, :w]) if bcast else tc_[:, :w],
                                 c_dst, AF.Tanh)
            t3 = tpool.tile([P, CHUNK], F32, tag="tmp")
            nc.vector.tensor_mul(t3[:, :w], go_[:, :w], tc_[:, :w])
            d_ = tpool.tile([P, CHUNK], F32, tag="tmp")
            nc.vector.tensor_sub(d_[:, :w], t3[:, :w], px[:, :w])
            e_ = tpool.tile([P, CHUNK], F32, tag="tmp")
            pool_e.tensor_mul(e_[:, :w], gr_[:, :w], d_[:, :w])
            if h_dst is not None:
                pool_e.tensor_add(h_dst,
                                  v3(e_[:, :w]) if bcast else e_[:, :w],
                                  v3(px[:, :w]) if bcast else px[:, :w])
            hf = opool.tile([P, CHUNK], F32, tag="hf")
            pool_e.tensor_add(hf[:, :w], e_[:, :w], px[:, :w])
            nc.sync.dma_start(outT_r[:, t, col0:col0 + w], hf[:, :w])

        def chunk(d, col0, p0, w, store, buf, wq0, b):
            """One chunk of w node-columns at level d (single side b).
            col0: featsT/outT column base; p0: parent position in parent state
            buffers; store: write c/h state; buf: this level's state buffer
            idx; wq0: within-seg parent offset for state writes; b: side."""
            pbuf = (d - 1) % 2
            ft = fpool.tile([P, KT, CHUNK], FP8, tag="feats")
            ftb = fbpool.tile([P, KT, CHUNK], BF16, tag="featsb")
            nc.sync.dma_start(ft[:, :, :w], featsT_r[:, :, col0:col0 + w])
            nc.sync.dma_start(ftb[:, :, :w], featsB_r[:, :, col0:col0 + w])
            for t in range(KT):
                ps_px = pspool.tile([P, CHUNK], F32, tag="ps")
                px_from(ps_px, t, ftb, w)
                gates = []
                for gi in range(5):  # i, o, f, u, r
                    m = gi * KT + t
                    ps = pspool.tile([P, CHUNK], F32, tag="ps")
                    mm8(ps[:, :w], wx_sb, m, ft, 0, w,
                        first=True, last=(d == 0))
                    if d > 0:
                        mm8(ps[:, :w], wh_sb, m, hst[pbuf], p0, w,
                            first=False, last=True)
                    g = gpool.tile([P, CHUNK], F32, tag="gates")
                    func = AF.Tanh if gi == 3 else AF.Sigmoid
                    nc.scalar.activation(g[:, :w], ps[:, :w], func,
                                         bias=bias_sb[:, m:m + 1], scale=inv_s)
                    gates.append(g)

                if store:
                    c_dst = cst[buf][:, t, 2 * wq0 + b: 2 * (wq0 + w) + b - 1: 2]
                    h_dst = hst[buf][:, t, 2 * wq0 + b: 2 * (wq0 + w) + b - 1: 2]
                else:
                    c_dst = tpool.tile([P, CHUNK], F32, tag="tmp",
                                       name="ctmp")[:, :w]
                    h_dst = None
                pc_ap = cst[pbuf][:, t, p0:p0 + w] if d > 0 else None
                elemwise(t, w, gates, ps_px, pc_ap, c_dst, h_dst, col0,
                         bcast=False)

        def chunk_merged(d, col0, w, store, buf):
            """Both halves of a full small level (w = N <= 512) in one chunk:
            x-GEMM over all w cols; the parent-h GEMM runs twice (once per
            output half, same stationary weights, same parent h slice) —
            avoiding broadcast moving APs which DoubleRow forbids."""
            pbuf = (d - 1) % 2
            half = w // 2
            ft = fpool.tile([P, KT, CHUNK], FP8, tag="feats")
            ftb = fbpool.tile([P, KT, CHUNK], BF16, tag="featsb")
            nc.sync.dma_start(ft[:, :, :w], featsT_r[:, :, col0:col0 + w])
            nc.sync.dma_start(ftb[:, :, :w], featsB_r[:, :, col0:col0 + w])
            for t in range(KT):
                ps_px = pspool.tile([P, CHUNK], F32, tag="ps")
                px_from(ps_px, t, ftb, w)
                gates = []
                for gi in range(5):
                    m = gi * KT + t
                    ps = pspool.tile([P, CHUNK], F32, tag="ps")
                    mm8(ps[:, :w], wx_sb, m, ft, 0, w, first=True, last=False)
                    for hb in range(2):
                        mm8(ps[:, hb * half:hb * half + half], wh_sb, m,
                            hst[pbuf], 0, half, first=False, last=(hb == 1))
                    g = gpool.tile([P, CHUNK], F32, tag="gates")
                    func = AF.Tanh if gi == 3 else AF.Sigmoid
                    nc.scalar.activation(g[:, :w], ps[:, :w], func,
                                         bias=bias_sb[:, m:m + 1], scale=inv_s)
                    gates.append(g)

                if store:
                    c_dst = cst[buf][:, t, 0:w].rearrange("p (q b) -> p b q",
                                                          b=2)
                    h_dst = hst[buf][:, t, 0:w].rearrange("p (q b) -> p b q",
                                                          b=2)
                else:
                    c_dst = v3(tpool.tile([P, CHUNK], F32, tag="tmp",
                                          name="ctmp2")[:, :w])
                    h_dst = None
                pc_ap = cst[pbuf][:, t, None, 0:half].to_broadcast((P, 2, half))
                elemwise(t, w, gates, ps_px, pc_ap, c_dst, h_dst, col0,
                         bcast=True)

        for (d, s, l) in segs:
            if d < HD:
                continue  # computed host-side
            store = d < depth - 1
            buf = d % 2
            parent_base = s // 2 if (d == depth - 1 and split) else 0
            if d == 0:
                chunk(0, off[0], 0, 1, store, buf, 0, 0)
                continue
            if l == Ns[d] and l <= CHUNK:
                chunk_merged(d, off[d], l, store, buf)
                continue
            plen = l // 2
            for q0 in range(0, plen, CHUNK):
                w = min(CHUNK, plen - q0)
                p0 = s // 2 + q0 - parent_base
                for b in (0, 1):
                    col0 = off[d] + b * (Ns[d] // 2) + s // 2 + q0
                    chunk(d, col0, p0, w, store, buf, q0, b)

    nc.compile()
    return nc, C


# ---------------------------------------------------------------- host side

def _col_maps(depth):
    """Per (core, level): global node indices for each comp-order column."""
    Ns = _level_sizes(depth)
    maps = []  # maps[core][level] -> np.int64 [N_d] global node idx per column
    for i in range(NCORES):
        per_level = []
        for d in range(depth):
            N = Ns[d]
            logical = np.concatenate([np.arange(0, N, 2), np.arange(1, N, 2)])
            if d <= 3:
                orig = logical ^ (i >> (3 - d))
            else:
                orig = i * (1 << (d - 3)) + logical
            per_level.append(((1 << d) - 1) + orig)
        maps.append(per_level)
    return maps


_HOST_OUT = {}


def prep_inputs(features, px_w, px_b, iofux_w, iofux_b, iofuh_w, iofuh_b,
                depth=DEPTH):
    Ns = _level_sizes(depth)
    C = sum(Ns)
    HD = _host_levels(depth)
    features = np.asarray(features, np.float32)
    px_w = np.asarray(px_w, np.float32)
    px_b = np.asarray(px_b, np.float32)
    iofux_w = np.asarray(iofux_w, np.float32)
    iofux_b = np.asarray(iofux_b, np.float32)
    iofuh_w = np.asarray(iofuh_w, np.float32)
    iofuh_b = np.asarray(iofuh_b, np.float32)
    wxT = np.ascontiguousarray(iofux_w.T * S_W).astype(np_fp8)   # [512, 2560]
    whT = np.ascontiguousarray(iofuh_w.T * S_W).astype(np_fp8)
    pxwT = np.ascontiguousarray(px_w.T).astype(np_bf16)          # [512, 512]
    bias_all = np.concatenate([iofux_b + iofuh_b, px_b])         # [3072]
    biasm = np.ascontiguousarray(bias_all.reshape(M_ALL, P).T)   # [128, 24]

    c_last = np.zeros((1, H), np.float32)
    h_last = np.zeros((1, H), np.float32)
    host_rows = np.zeros((max(1, (1 << HD) - 1), H), np.float32)
    sig = lambda x: 1.0 / (1.0 + np.exp(-x))
    for d in range(HD):
        s0, n = (1 << d) - 1, (1 << d)
        feats = features[s0:s0 + n]
        pc = c_last if d == 0 else np.repeat(c_last, 2, axis=0)
        ph = h_last if d == 0 else np.repeat(h_last, 2, axis=0)
        px = feats @ px_w.T + px_b
        iofu = feats @ iofux_w.T + iofux_b + ph @ iofuh_w.T + iofuh_b
        ii, oo, ff, uu, rr = np.split(iofu, 5, axis=1)
        ii, oo, ff, rr = sig(ii), sig(oo), sig(ff), sig(rr)
        uu = np.tanh(uu)
        cc = ii * uu + ff * pc
        hh = oo * np.tanh(cc)
        hf = rr * (hh - px) + px
        host_rows[s0:s0 + n] = hf
        c_last, h_last = cc, hf
    _HOST_OUT[depth] = host_rows

    maps = _col_maps(depth)
    in_maps = []
    for i in range(NCORES):
        cols = np.concatenate(maps[i])                           # [C]
        fcore = features[cols, :]                                # [C, 512] f32
        fT32 = np.ascontiguousarray(fcore.T)                     # [512, C]
        m = {"featsT": fT32.astype(np_fp8),
             "featsB": fT32.astype(np_bf16),
             "wxT": wxT, "whT": whT, "pxwT": pxwT,
             "biasm": biasm}
        if HD:
            m["c_init"] = np.ascontiguousarray(
                c_last[i][:, None]).astype(np.float32)
            m["h_init"] = np.ascontiguousarray(
                h_last[i][:, None]).astype(np_fp8)
        in_maps.append(m)
    return in_maps, maps, C


def assemble_output(results, maps, depth=DEPTH):
    Ns = _level_sizes(depth)
    HD = _host_levels(depth)
    n_nodes = (1 << depth) - 1
    out = np.empty((n_nodes, H), np.float32)
    offs = np.cumsum([0] + Ns)
    if HD:
        out[: (1 << HD) - 1] = _HOST_OUT[depth]
    for i in range(NCORES):
        o = results[i]["outT"]                                   # [512, C] f32
        for d in range(HD, depth):
            if d <= 3 and i != 0:
                continue  # replicated levels: take core 0's copy
            cols = maps[i][d]
            out[cols, :] = o[:, offs[d]:offs[d + 1]].T
    return out


_CACHE = {}


def _get_built(depth=DEPTH):
    if depth not in _CACHE:
        _CACHE[depth] = build_nc(depth)
    return _CACHE[depth]


def run_cores(in_maps, depth=DEPTH, trace=False):
    from concourse.bass_utils import run_bass_kernel_spmd
    nc, C = _get_built(depth)
    br = run_bass_kernel_spmd(nc, in_maps, list(range(NCORES)), trace=trace)
    return br


def kernel(features, px_w, px_b, iofux_w, iofux_b, iofuh_w, iofuh_b):
    in_maps, maps, C = prep_inputs(features, px_w, px_b, iofux_w, iofux_b,
                                   iofuh_w, iofuh_b)
    br = run_cores(in_maps)
    return assemble_output(br.results, maps)
